# revision 31
# baseline (speedup 1.0000x reference)
"""Trainium2 Bass kernel: multi-head attention (B=4, N=1024, D=1024, H=16)
distributed over 8 NeuronCores.

Sharding: core = (batch b, head-group hg), hg selecting 8 of the 16 heads.
Each core projects Q/K/V for its 8 heads only (column-parallel w_qkv), runs
attention for those heads over all 1024 queries, and applies the
row-parallel slice of w_out, producing a partial y[1024, 1024] (fp32).
The host sums the two partials per batch and adds the bias.  This removes
the duplicated K/V projection work of a batch/query-half sharding (~20% of
PE columns).

Per-core schedule: after a short preamble (Q/K for head pair 0), head
iterations are software-pipelined: iteration h emits scores(h+1)
interleaved with PV(h) (lagging 3 score-tiles so the deferred
normalization of head h-1 can free the PV psum banks first) while the
Scalar engine runs the exp stream.  Softmax denominators come free from a
ones-column appended to V; their reciprocal runs on the Scalar engine
([1,512] reciprocal costs ~0.7us there vs 3.4us on DVE).
"""

import numpy as np
import concourse.bacc as bacc
import concourse.mybir as mybir
import concourse.tile as tile

dt = mybir.dt
F32, BF16 = dt.float32, dt.bfloat16

B, N, D = 4, 1024, 1024
H, DH = 16, 64
HG = 8              # heads per core
DG = HG * DH        # 512 head dims per core
P = 128
DC = D // P         # 8 contraction chunks over D
NT = N // P         # 8 key-token tiles
ET = DG // P        # 4 feature tiles (head pairs)
SCALE = DH ** -0.5
AF = mybir.ActivationFunctionType


def _build_nc():
    nc = bacc.Bacc("TRN2", target_bir_lowering=False, debug=False)
    # wqT/wkT come pre-chunked by head-pair ([ET, D, P]) so the first tile's
    # upload is small and contiguous: the PE can start the Q projection ~6us
    # after launch instead of waiting out one big strided transfer.
    xT = nc.dram_tensor("xT", [D, N], BF16, kind="ExternalInput")
    wqT = nc.dram_tensor("wqT", [ET, D, P], BF16, kind="ExternalInput")
    wkT = nc.dram_tensor("wkT", [ET, D, P], BF16, kind="ExternalInput")
    wvT = nc.dram_tensor("wvT", [D, DG], BF16, kind="ExternalInput")
    wo = nc.dram_tensor("wo", [DG, D], BF16, kind="ExternalInput")
    y = nc.dram_tensor("y", [N, D], BF16, kind="ExternalOutput")

    with tile.TileContext(nc) as tc:
        with (
            tc.tile_pool(name="const", bufs=1) as cp,
            tc.tile_pool(name="work", bufs=2) as wp,
            tc.tile_pool(name="ps", bufs=1, space="PSUM") as pp,
        ):
            xT_sb = cp.tile([P, DC, N], BF16)
            wq_sb = cp.tile([P, DC, DG], BF16)
            wk_sb = cp.tile([P, DC, DG], BF16)
            wv_sb = cp.tile([P, DC, DG], BF16)
            wo_sb = cp.tile([P, ET, D], BF16)

            # DMA order = consumption order: Q/K proj inputs for head pair 0
            # first, split into small transfers that fan out over the DMA
            # rings so the first chain's chunks land early.
            # Interleave the three critical streams chunk-by-chunk so the
            # round-robin queue assignment lands every chunk of the first
            # Q/K chains' operands in the first parallel wave.
            for c in range(DC):
                nc.sync.dma_start(wq_sb[:, c, 0:P],
                                  wqT.ap()[0][c * P:(c + 1) * P, :])
                nc.sync.dma_start(xT_sb[:, c, 0:512],
                                  xT.ap()[c * P:(c + 1) * P, 0:512])
                nc.sync.dma_start(wk_sb[:, c, 0:P],
                                  wkT.ap()[0][c * P:(c + 1) * P, :])
            for c in range(DC):
                nc.sync.dma_start(xT_sb[:, c, 512:N],
                                  xT.ap()[c * P:(c + 1) * P, 512:N])
            for et in range(1, ET):
                nc.sync.dma_start(wq_sb[:, :, et * P:(et + 1) * P],
                                  wqT.ap()[et].rearrange("(c p) e -> p c e", p=P))
                nc.sync.dma_start(wk_sb[:, :, et * P:(et + 1) * P],
                                  wkT.ap()[et].rearrange("(c p) e -> p c e", p=P))
            nc.sync.dma_start(wv_sb[:, :, :],
                              wvT.ap().rearrange("(c p) e -> p c e", p=P))
            nc.sync.dma_start(wo_sb[:, :, :],
                              wo.ap().rearrange("(c p) e -> p c e", p=P))

            q_sb = cp.tile([P, ET, N], BF16)
            k_sb = cp.tile([P, ET, N], BF16)
            # V stationary tile is 128 wide: a ones column at index 0 puts the
            # softmax denominator in psum row 0 (reciprocal_approx_fast, a
            # custom DVE op, drops input partition offsets, and psum reads
            # must start at a multiple of 32 anyway); the V dims sit at
            # columns 64..127 so the normalize multiply reads psum rows
            # 64..127 (offset 64 is legal).  Columns 1..63 are zeroed.
            v_sb = cp.tile([P, NT, HG, P], BF16)
            nc.vector.memset(v_sb[:, :, :, 0:1], 1.0)
            nc.vector.memset(v_sb[:, :, :, 1:DH], 0.0)
            aT_sb = cp.tile([P, ET, N], BF16)

            # ---- projection chains, exposed as single-matmul filler steps ----
            def qk_steps(w_sb, out_sb, et, j, pfx):
                st = {}
                def step(c):
                    if c == 0:
                        st["ps"] = pp.tile([P, 512], F32, tag="proj", bufs=2,
                                           name=f"{pfx}{et}_{j}")
                    nc.tensor.matmul(
                        st["ps"][:, :],
                        lhsT=w_sb[:, c, et * P:(et + 1) * P],
                        rhs=xT_sb[:, c, j * 512:(j + 1) * 512],
                        start=(c == 0), stop=(c == DC - 1),
                    )
                    if c == DC - 1:
                        nc.vector.tensor_copy(out_sb[:, et, j * 512:(j + 1) * 512],
                                              st["ps"][:, :])
                return [lambda c=c: step(c) for c in range(DC)]

            def q_steps(et, j):
                return qk_steps(wq_sb, q_sb, et, j, "qps")

            def k_steps(et, j):
                return qk_steps(wk_sb, k_sb, et, j, "kps")

            def v_steps(nt):
                st = {}
                def step(c):
                    if c == 0:
                        st["ps"] = pp.tile([P, DG], F32, tag="proj", bufs=2,
                                           name=f"vps{nt}")
                    nc.tensor.matmul(
                        st["ps"][:, :],
                        lhsT=xT_sb[:, c, nt * P:(nt + 1) * P],
                        rhs=wv_sb[:, c, :],
                        start=(c == 0), stop=(c == DC - 1),
                    )
                    if c == DC - 1:
                        nc.vector.tensor_copy(
                            v_sb[:, nt, :, DH:P],
                            st["ps"][:, :].rearrange("p (h d) -> p h d", h=HG),
                        )
                return [lambda c=c: step(c) for c in range(DC)]

            # Fillers threaded between attention matmuls so the in-order PE
            # queue stays busy while the Scalar exp stream catches up.
            # late list: Q/K for head pair 3 (not needed until iteration 5's
            # scores(6)) rides inside iterations 0-4.
            # Q/K for pairs 1-2 first (their weights land right after xT), V
            # chains after (wv is queued behind the Q/K weight uploads).
            fillers = []
            for et in (1, 2):
                for j in range(2):
                    fillers += q_steps(et, j)
                for j in range(2):
                    fillers += k_steps(et, j)
            for nt in range(NT):
                fillers += v_steps(nt)
            late = []
            for j in range(2):
                late += k_steps(3, j)
            for j in range(2):
                late += q_steps(3, j)
            fill_pos = [0]
            late_pos = [0]

            def pop_filler(n):
                k = 0
                while k < n and fill_pos[0] < len(fillers):
                    fillers[fill_pos[0]]()
                    fill_pos[0] += 1
                    k += 1

            def pop_late(n):
                k = 0
                while k < n and late_pos[0] < len(late):
                    late[late_pos[0]]()
                    late_pos[0] += 1
                    k += 1

            state = {}

            def new_head(h):
                state[h] = {"pT": wp.tile([P, NT, 2, 512], BF16, tag="pT",
                                          bufs=3, name=f"pT{h}"),
                            "pv": {}}

            def s_tile(h, j, c):
                t, r = h // 2, (h % 2) * DH
                s_ps = pp.tile([P, 512], F32, tag="s", bufs=2, name=f"s{h}_{j}_{c}")
                nc.tensor.matmul(
                    s_ps[:, :],
                    lhsT=k_sb[r:r + DH, t, c * P:(c + 1) * P],
                    rhs=q_sb[r:r + DH, t, j * 512:(j + 1) * 512],
                    start=True, stop=True,
                )
                nc.scalar.activation(state[h]["pT"][:, c, j, :], s_ps[:, :],
                                     AF.Exp, scale=SCALE)

            def pv_link(h, j, c):
                st = state[h]
                if c == 0:
                    st["pv"][j] = pp.tile([P, 512], F32, tag="pv", bufs=2,
                                          name=f"pv{h}_{j}")
                nc.tensor.matmul(
                    st["pv"][j][:, :],
                    lhsT=v_sb[:, c, h, :],
                    rhs=st["pT"][:, c, j, :],
                    start=(c == 0), stop=(c == NT - 1),
                )
                if c == NT - 1:
                    # Normalization, inline as the chain closes:
                    # 1/s on DVE (fast-approx, 18 bits), broadcast over the 64
                    # head dims on the idle GpSimd engine, multiply on DVE.
                    # No PE or Scalar involvement; frees the pv bank promptly.
                    t, r = h // 2, (h % 2) * DH
                    srec = st.setdefault(
                        "srec", wp.tile([1, 2, 512], F32, tag="srec", bufs=2,
                                        name=f"sr{h}"))
                    nc.vector.reciprocal_approx_fast(srec[:, j, :],
                                                     st["pv"][j][0:1, :])
                    bc_sb = wp.tile([DH, 512], F32, tag="bc_sb", bufs=4,
                                    name=f"bcs{h}_{j}")
                    nc.gpsimd.partition_broadcast(bc_sb[:, :], srec[:, j, :])
                    nc.vector.tensor_mul(aT_sb[r:r + DH, t, j * 512:(j + 1) * 512],
                                         st["pv"][j][DH:P, :], bc_sb[:, :])

            # ---- emission ----
            # PE warmup: the Tensor engine clock ramps to full speed only
            # after ~3us of continuous execution.  Stream dummy matmuls (into
            # a never-read psum tile) while the input DMAs land so the real
            # matmuls start at full clock instead of paying the 2.7x-slow
            # pstate ramp.
            scratch = cp.tile([1, 512], BF16)
            nc.vector.memset(scratch, 0.0)
            for i in range(10):
                warm_ps = pp.tile([DH, 512], F32, tag="s", bufs=2,
                                  name=f"warm{i}")
                nc.tensor.matmul(warm_ps[:, :], lhsT=scratch[:, 0:DH],
                                 rhs=scratch[:, :], start=True, stop=True)

            # Preamble: Q/K for head pair 0, then scores for heads 0 AND 1
            # threaded with projection fillers (the Scalar exp stream for two
            # heads overlaps the PE-bound projection stretch), then the
            # remaining projections.
            for s in q_steps(0, 0) + q_steps(0, 1) + k_steps(0, 0) + k_steps(0, 1):
                s()
            PRE_S = 1
            for hh in range(PRE_S):
                new_head(hh)
                for j in range(2):
                    for c in range(NT):
                        s_tile(hh, j, c)
                        pop_filler(2)
            pop_filler(len(fillers))

            # Head iterations.  Iteration h: scores(h+PRE_S) tiles drive;
            # pv(h) links lag LAG tiles behind (head h's pv banks are freed
            # by the inline normalization shortly after each chain closes).
            LAG = 2
            for h in range(HG):
                nxt = h + PRE_S
                slots = []
                if nxt < HG:
                    new_head(nxt)
                    slots += [("s", nxt, j, c) for j in range(2) for c in range(NT)]
                pv_slots = [("pv", h, j, c) for j in range(2) for c in range(NT)]
                out = []
                for i, sl in enumerate(slots):
                    out.append(sl)
                    if i >= LAG - 1 and i - (LAG - 1) < len(pv_slots):
                        out.append(pv_slots[i - (LAG - 1)])
                n_done = max(len(slots) - (LAG - 1), 0) if slots else 0
                out += pv_slots[n_done:]
                for sl in out:
                    if sl[0] == "s":
                        s_tile(*sl[1:])
                        if h < 4:
                            pop_late(1)
                    else:
                        pv_link(*sl[1:])

            # Output projection: y partial [1024, 1024], contracting the 4
            # feature tiles (all 8 heads of this group).  Partials go out in
            # bf16 (the host sums in fp32; well within the error budget) via
            # small DMAs fanned over the rings so the final transfer is short.
            # The proj/s/pv psum tags are idle by now: alternate y chains over
            # the "s" and "bcy" tags (4 banks in flight) and alternate the
            # evictions between Scalar and Vector so bank recycling never
            # gates the PE.
            for qt in range(N // P):
                y_sb = wp.tile([P, D], BF16, tag="y_sb", bufs=2, name=f"ysb{qt}")
                for jE in range(2):
                    y_ps = pp.tile([P, 512], F32, tag=("bcy" if jE else "s"),
                                   bufs=2, name=f"yps{qt}_{jE}")
                    for et in range(ET):
                        nc.tensor.matmul(
                            y_ps[:, :],
                            lhsT=aT_sb[:, et, qt * P:(qt + 1) * P],
                            rhs=wo_sb[:, et, jE * 512:(jE + 1) * 512],
                            start=(et == 0), stop=(et == ET - 1),
                        )
                    if jE:
                        nc.vector.tensor_copy(y_sb[:, 512:1024], y_ps[:, :])
                    else:
                        nc.scalar.activation(y_sb[:, 0:512], y_ps[:, :], AF.Copy)
                    nc.sync.dma_start(
                        y.ap()[qt * P:(qt + 1) * P, jE * 512:(jE + 1) * 512],
                        y_sb[:, jE * 512:(jE + 1) * 512])
    nc.compile()
    return nc


def _make_in_maps(x, w_qkv, w_out, b_out):
    import ml_dtypes
    bf = ml_dtypes.bfloat16
    wq, wk, wv = w_qkv[0:D], w_qkv[D:2 * D], w_qkv[2 * D:3 * D]
    in_maps = []
    for core in range(8):
        b, hg = core // 2, core % 2
        s = slice(hg * DG, (hg + 1) * DG)
        # wqT/wkT pre-chunked by head pair: [ET, D, P]
        wqTs = np.ascontiguousarray(
            wq[s].astype(bf).T.reshape(D, ET, P).transpose(1, 0, 2))
        wkTs = np.ascontiguousarray(
            wk[s].astype(bf).T.reshape(D, ET, P).transpose(1, 0, 2))
        in_maps.append({
            "xT": np.ascontiguousarray(x[b].astype(bf).T),
            "wqT": wqTs,
            "wkT": wkTs,
            "wvT": np.ascontiguousarray(wv[s].astype(bf).T),
            # reference einsum is 'bnd,ed->bne': w_out columns are the
            # attention-dim (contraction) axis, so the row-parallel slice is
            # columns hg*DG:(hg+1)*DG of w_out, transposed to [DG, D].
            "wo": np.ascontiguousarray(w_out[:, s].T.astype(bf)),
        })
    return in_maps


def _assemble(results, b_out):
    y = np.empty((B, N, D), dtype=np.float32)
    for b in range(B):
        y[b] = (results[2 * b]["y"].astype(np.float32)
                + results[2 * b + 1]["y"].astype(np.float32))
    y += b_out.astype(np.float32)
    return y


_NC_CACHE = {}


def kernel(x, w_qkv, w_out, b_out):
    import numpy as _np
    from concourse.bass_utils import run_bass_kernel_spmd
    if "nc" not in _NC_CACHE:
        _NC_CACHE["nc"] = _build_nc()
    nc = _NC_CACHE["nc"]
    in_maps = _make_in_maps(_np.asarray(x), _np.asarray(w_qkv),
                            _np.asarray(w_out), _np.asarray(b_out))
    res = run_bass_kernel_spmd(nc, in_maps, list(range(8)))
    return _assemble(res.results, _np.asarray(b_out))
